# revision 21
# baseline (speedup 1.0000x reference)
"""Trainium2 Bass kernel: ContextCrossAttention (B,C,H,W)=(8,512,128,128).

Math per batch element b (algebraically collapsed from the reference):
  q      = Wq @ ctx_b + bq                          (C,)
  qks    = (q @ Wk) * C**-0.5                       (C,)     # logits = qks . x[:, hw] (+ shift, dropped)
  p[hw]  = exp(logits[hw]);  Z = sum(p)                      # softmax shift-invariance: no max-subtract
  pooled = x_b @ p                                  (C,)
  gate   = (Wv @ pooled) / Z + bv                   (C,)
  out_b  = x_b * gate[:, None]

Sharding: pure data-parallel over batch; core i handles batch element i.

The kernel is HBM-bound, so x is streamed in bf16 (host-side downcast):
16 MiB/core instead of 32, which also lets the whole x reside in SBUF --
pass C (out = x * gate) re-reads nothing.  The output is stored as bf16
and upcast on the host.  All error terms stay ~1e-3 relative.

Measured engine facts baked into the structure:
  - STT (x*p with accum) on DVE runs at the 1x roofline (2.3us per
    [128,2048]) ONLY when gpsimd is quiet; a concurrent gpsimd multiply
    poisons it 2.6x.  So gpsimd does NOT participate in the pooled pass.
  - The logits matmul uses a column-replicated stationary (qksB), so PSUM
    logits land on all 128 partitions and exp() emits p already
    partition-broadcast; Z needs no extra broadcast either.
  - Weights load as one chunk-major DMA each on the scalar queue, in
    parallel with the x stream on the sync queue (startup latency).
"""

import numpy as np
import ml_dtypes
from contextlib import ExitStack

import concourse.bass as bass
import concourse.bacc as bacc
import concourse.tile as tile
from concourse import mybir
from concourse.bass_utils import run_bass_kernel_spmd

F32 = mybir.dt.float32
BF16 = mybir.dt.bfloat16
AF = mybir.ActivationFunctionType
OP = mybir.AluOpType

B, C, D, H, W = 8, 512, 512, 128, 128
HW = H * W                      # 16384
P = 128                         # partitions
CCH = C // P                    # 4 channel chunks
NCORES = 8
G = 8                           # hw groups
GW = HW // G                    # 2048 group width
SCALE = float(C) ** -0.5

PGW = 1024                      # psum logits group width (2 banks each)
NH = GW // PGW                  # 2 psum halves per group


def _build_kernel():
    nc = bacc.Bacc(
        "TRN2",
        target_bir_lowering=False,
        debug=False,
        enable_asserts=False,
        num_devices=NCORES,
    )

    xd = nc.dram_tensor("xb", [C, HW], BF16, kind="ExternalInput")
    ctxd = nc.dram_tensor("ctxc", [P, CCH], BF16, kind="ExternalInput")   # ctx[j*128+p] at [p, j]
    # weights are pre-chunked on the host: w_c[p, j*C + k] = W[j*128 + p, k]
    # wqk = Wq.T @ Wk * scale (host-folded), bqk = bq @ Wk * scale
    wqkd = nc.dram_tensor("wqk", [P, CCH * C], BF16, kind="ExternalInput")
    wvtd = nc.dram_tensor("wvt", [P, CCH * C], BF16, kind="ExternalInput")
    bqkd = nc.dram_tensor("bqkc", [P, CCH], F32, kind="ExternalInput")
    bvd = nc.dram_tensor("bvc", [P, CCH], F32, kind="ExternalInput")
    outd = nc.dram_tensor("out", [C, HW], BF16, kind="ExternalOutput")

    with tile.TileContext(nc) as tc, ExitStack() as ctx:
        singles = ctx.enter_context(tc.tile_pool(name="singles", bufs=1))
        xt = ctx.enter_context(tc.tile_pool(name="xt", bufs=G * CCH))
        scr = ctx.enter_context(tc.tile_pool(name="scr", bufs=1))
        scra = ctx.enter_context(tc.tile_pool(name="scra", bufs=2))
        prods = ctx.enter_context(tc.tile_pool(name="prods", bufs=2))
        outp = ctx.enter_context(tc.tile_pool(name="outp", bufs=4))
        psb = ctx.enter_context(tc.tile_pool(name="psb", bufs=3))
        pslog = ctx.enter_context(tc.tile_pool(name="pslog", bufs=3, space="PSUM"))
        pssm = ctx.enter_context(tc.tile_pool(name="pssm", bufs=2, space="PSUM"))

        # ---- prefix loads, AHEAD of the x stream on the sync queue: the
        # tiny tensors first, then the two weight matrices the logits
        # stationary depends on.  (One chunk-major DMA per weight.) ----
        wqk_sb = singles.tile([P, CCH * C], BF16, tag="wqk", name="wqk")
        nc.sync.dma_start(wqk_sb[:], wqkd[:])
        ctx_sb = singles.tile([P, CCH], BF16, tag="ctx", name="ctx")
        nc.scalar.dma_start(ctx_sb[:], ctxd[:])
        bqk_sb = singles.tile([P, CCH], F32, tag="bqk", name="bqk")
        nc.scalar.dma_start(bqk_sb[:], bqkd[:])

        ones128 = singles.tile([P, P], BF16, tag="ones128")
        nc.vector.memset(ones128[:], 1.0)

        qks_sb = singles.tile([P, CCH], F32, tag="qks")
        qksb = [singles.tile([P, P], BF16, tag=f"qksb{cc}", name=f"qksb{cc}") for cc in range(CCH)]
        pooled_sb = singles.tile([P, CCH], BF16, tag="pooled")
        pooled_f32 = singles.tile([P, CCH], F32, tag="pooledf")
        gate_sb = singles.tile([P, CCH], F32, tag="gate")
        zcols = singles.tile([P, G * NH], F32, tag="zcols")
        pcols = [singles.tile([P, G], F32, tag=f"pcols{cc}", name=f"pcols{cc}") for cc in range(CCH)]
        z_sb = singles.tile([P, 1], F32, tag="z")
        rz_sb = singles.tile([P, 1], F32, tag="rz")

        # ---- qks = ctx @ Wqk + bqk (pre-scaled); qksB replicated 128x ----
        for cc in range(CCH):
            pqk = pssm.tile([P, 1], F32, tag="pssm", name="pssm_t")
            for dc in range(CCH):
                nc.tensor.matmul(
                    pqk[:], wqk_sb[:, dc * C + cc * P:dc * C + (cc + 1) * P],
                    ctx_sb[:, dc:dc + 1],
                    start=(dc == 0), stop=(dc == CCH - 1),
                )
            nc.vector.tensor_add(qks_sb[:, cc:cc + 1], pqk[:], bqk_sb[:, cc:cc + 1])
            nc.vector.tensor_scalar_mul(qksb[cc][:], ones128[:], qks_sb[:, cc:cc + 1])

        # ---- fused pass A+B: logits (all-partition rows) -> exp -> pooled ----
        # DVE: 3 fused STTs + 1 TT-mult per group; ACT: 2 exps + 1 reduce
        # (lagged one group so exps never queue behind it).
        x_tiles = {}
        pr_tiles = {}

        def _act_reduce(g):
            for cc in (2, 3):
                sa = scra.tile([P, GW], BF16, tag="scra", name="scra_t")
                nc.scalar.activation(
                    sa[:], pr_tiles[(cc, g)][:], AF.Copy,
                    accum_out=pcols[cc][:, g:g + 1],
                )

        for g in range(G):
            for cc in range(CCH):
                t = xt.tile([P, GW], BF16, tag="x", name="x_t")
                nc.sync.dma_start(t[:], xd[cc * P:(cc + 1) * P, g * GW:(g + 1) * GW])
                x_tiles[(cc, g)] = t
            p_t = psb.tile([P, GW], BF16, tag="p", name="p_t")
            for h in range(NH):
                gh = g * NH + h
                plog = pslog.tile([P, PGW], F32, tag="plog", name="plog_t")
                for s in range(PGW // 512):
                    for cc in range(CCH):
                        nc.tensor.matmul(
                            plog[:, s * 512:(s + 1) * 512],
                            qksb[cc][:],
                            x_tiles[(cc, g)][:, h * PGW + s * 512:h * PGW + (s + 1) * 512],
                            start=(cc == 0), stop=(cc == CCH - 1),
                        )
                nc.scalar.activation(
                    p_t[:, h * PGW:(h + 1) * PGW], plog[:], AF.Exp,
                    accum_out=zcols[:, gh:gh + 1],
                )
            if g > 0:
                _act_reduce(g - 1)
            for cc in (0, 1):
                sc = scr.tile([P, GW], BF16, tag="scrv", name="scr_t")
                nc.vector.scalar_tensor_tensor(
                    sc[:], x_tiles[(cc, g)][:], 1.0, p_t[:],
                    op0=OP.mult, op1=OP.mult,
                    accum_out=pcols[cc][:, g:g + 1],
                )
            for cc in (2, 3):
                pr = prods.tile([P, GW], BF16, tag=f"pr{cc}", name=f"pr{cc}_t")
                nc.vector.tensor_mul(pr[:], x_tiles[(cc, g)][:], p_t[:])
                pr_tiles[(cc, g)] = pr
        _act_reduce(G - 1)

        # ---- late loads: Wv.T / bv, queued behind the x stream ----
        wvt_sb = singles.tile([P, CCH * C], BF16, tag="wvt", name="wvt")
        nc.scalar.dma_start(wvt_sb[:], wvtd[:])
        bv_sb = singles.tile([P, CCH], F32, tag="bv", name="bv")
        nc.scalar.dma_start(bv_sb[:], bvd[:])

        # ---- finalize: Z, pooled, gate = (Wv @ pooled)/Z + bv ----
        nc.vector.reduce_sum(z_sb[:], zcols[:], axis=mybir.AxisListType.X)
        nc.vector.reciprocal(rz_sb[:], z_sb[:])
        for cc in range(CCH):
            nc.vector.reduce_sum(
                pooled_f32[:, cc:cc + 1], pcols[cc][:], axis=mybir.AxisListType.X
            )
        nc.vector.tensor_copy(pooled_sb[:], pooled_f32[:])
        for oc in range(CCH):
            pg = pssm.tile([P, 1], F32, tag="pssm", name="pssm_t")
            for cc in range(CCH):
                nc.tensor.matmul(
                    pg[:], wvt_sb[:, cc * C + oc * P:cc * C + (oc + 1) * P],
                    pooled_sb[:, cc:cc + 1],
                    start=(cc == 0), stop=(cc == CCH - 1),
                )
            nc.vector.scalar_tensor_tensor(
                gate_sb[:, oc:oc + 1], pg[:], rz_sb[:], bv_sb[:, oc:oc + 1],
                op0=OP.mult, op1=OP.add,
            )

        # ---- pass C: out = x * gate (all of x is still resident in SBUF) ----
        for idx in range(G * CCH):
            g, cc = divmod(idx, CCH)
            o = outp.tile([P, GW], BF16, tag="o", name="o_t")
            nc.vector.tensor_scalar_mul(o[:], x_tiles[(cc, g)][:], gate_sb[:, cc:cc + 1])
            eng = nc.sync if idx % 2 == 0 else nc.scalar
            eng.dma_start(outd[cc * P:(cc + 1) * P, g * GW:(g + 1) * GW], o[:])

    nc.compile()
    return nc


_NC = None


def _get_nc():
    global _NC
    if _NC is None:
        _NC = _build_kernel()
    return _NC


def _chunk_major(w):
    # w_c[p, j*C + k] = w[j*128 + p, k]
    w = np.asarray(w, dtype=np.float32).reshape(CCH, P, C)
    return np.ascontiguousarray(w.transpose(1, 0, 2).reshape(P, CCH * C))


def _make_in_maps(x, context, Wq, bq, Wk, bk, Wv, bv):
    bf = ml_dtypes.bfloat16
    x = np.asarray(x, dtype=np.float32).reshape(B, C, HW).astype(bf)
    Wq = np.asarray(Wq, dtype=np.float32)
    Wk = np.asarray(Wk, dtype=np.float32)
    wqk = _chunk_major(Wq.T @ Wk * SCALE).astype(bf)
    bqk = (np.asarray(bq, dtype=np.float32) @ Wk) * SCALE
    wvt = _chunk_major(np.asarray(Wv, dtype=np.float32).T).astype(bf)
    bqkc = np.ascontiguousarray(bqk.reshape(CCH, P).T)
    bvc = np.ascontiguousarray(np.asarray(bv, dtype=np.float32).reshape(CCH, P).T)
    context = np.asarray(context, dtype=np.float32)
    in_maps = []
    for b in range(NCORES):
        ctxc = np.ascontiguousarray(context[b].reshape(CCH, P).T).astype(bf)
        in_maps.append({
            "xb": x[b],
            "ctxc": ctxc,
            "wqk": wqk,
            "wvt": wvt,
            "bqkc": bqkc,
            "bvc": bvc,
        })
    return in_maps


def run_spmd(x, context, Wq, bq, Wk, bk, Wv, bv, **spmd_kwargs):
    """Run on 8 NeuronCores; returns (output (B,C,H,W) f32, BassKernelResults)."""
    nc = _get_nc()
    in_maps = _make_in_maps(x, context, Wq, bq, Wk, bk, Wv, bv)
    res = run_bass_kernel_spmd(nc, in_maps, list(range(NCORES)), **spmd_kwargs)
    out = np.stack([
        np.asarray(res.results[b]["out"]).astype(np.float32).reshape(C, H, W)
        for b in range(NCORES)
    ])
    return out, res


def kernel(x, context, Wq, bq, Wk, bk, Wv, bv):
    out, _ = run_spmd(x, context, Wq, bq, Wk, bk, Wv, bv)
    return out


# revision 22
# speedup vs baseline: 1.0688x; 1.0688x over previous
"""Trainium2 Bass kernel: ContextCrossAttention (B,C,H,W)=(8,512,128,128).

Math per batch element b (algebraically collapsed from the reference):
  q      = Wq @ ctx_b + bq                          (C,)
  qks    = (q @ Wk) * C**-0.5                       (C,)     # logits = qks . x[:, hw] (+ shift, dropped)
  p[hw]  = exp(logits[hw]);  Z = sum(p)                      # softmax shift-invariance: no max-subtract
  pooled = x_b @ p                                  (C,)
  gate   = (Wv @ pooled) / Z + bv                   (C,)
  out_b  = x_b * gate[:, None]

Sharding: pure data-parallel over batch; core i handles batch element i.

The kernel is HBM-bound, so x is streamed in bf16 (host-side downcast):
16 MiB/core instead of 32, which also lets the whole x reside in SBUF --
pass C (out = x * gate) re-reads nothing.  The output is stored as bf16
and upcast on the host.  All error terms stay ~1e-3 relative.

Measured engine facts baked into the structure:
  - STT (x*p with accum) on DVE runs at the 1x roofline (2.3us per
    [128,2048]) ONLY when gpsimd is quiet; a concurrent gpsimd multiply
    poisons it 2.6x.  So gpsimd does NOT participate in the pooled pass.
  - The logits matmul uses a column-replicated stationary (qksB), so PSUM
    logits land on all 128 partitions and exp() emits p already
    partition-broadcast; Z needs no extra broadcast either.
  - Weights load as one chunk-major DMA each on the scalar queue, in
    parallel with the x stream on the sync queue (startup latency).
"""

import numpy as np
import ml_dtypes
from contextlib import ExitStack

import concourse.bass as bass
import concourse.bacc as bacc
import concourse.tile as tile
from concourse import mybir
from concourse.bass_utils import run_bass_kernel_spmd

F32 = mybir.dt.float32
BF16 = mybir.dt.bfloat16
AF = mybir.ActivationFunctionType
OP = mybir.AluOpType

B, C, D, H, W = 8, 512, 512, 128, 128
HW = H * W                      # 16384
P = 128                         # partitions
CCH = C // P                    # 4 channel chunks
NCORES = 8
G = 8                           # hw groups
GW = HW // G                    # 2048 group width
SCALE = float(C) ** -0.5

PGW = 1024                      # psum logits group width (2 banks each)
NH = GW // PGW                  # 2 psum halves per group


def _build_kernel():
    nc = bacc.Bacc(
        "TRN2",
        target_bir_lowering=False,
        debug=False,
        enable_asserts=False,
        num_devices=NCORES,
    )

    xd = nc.dram_tensor("xb", [C, HW], BF16, kind="ExternalInput")
    ctxd = nc.dram_tensor("ctxc", [P, CCH], BF16, kind="ExternalInput")   # ctx[j*128+p] at [p, j]
    # weights are pre-chunked on the host: w_c[p, j*C + k] = W[j*128 + p, k]
    # wqk = Wq.T @ Wk * scale (host-folded), bqk = bq @ Wk * scale
    wqkd = nc.dram_tensor("wqk", [P, CCH * C], BF16, kind="ExternalInput")
    wvtd = nc.dram_tensor("wvt", [P, CCH * C], BF16, kind="ExternalInput")
    bqkd = nc.dram_tensor("bqkc", [P, CCH], F32, kind="ExternalInput")
    bvd = nc.dram_tensor("bvc", [P, CCH], F32, kind="ExternalInput")
    outd = nc.dram_tensor("out", [C, HW], BF16, kind="ExternalOutput")

    with tile.TileContext(nc) as tc, ExitStack() as ctx:
        singles = ctx.enter_context(tc.tile_pool(name="singles", bufs=1))
        xt = ctx.enter_context(tc.tile_pool(name="xt", bufs=G * CCH))
        scr = ctx.enter_context(tc.tile_pool(name="scr", bufs=1))
        scra = ctx.enter_context(tc.tile_pool(name="scra", bufs=2))
        prods = ctx.enter_context(tc.tile_pool(name="prods", bufs=2))
        outp = ctx.enter_context(tc.tile_pool(name="outp", bufs=4))
        psb = ctx.enter_context(tc.tile_pool(name="psb", bufs=3))
        pslog = ctx.enter_context(tc.tile_pool(name="pslog", bufs=3, space="PSUM"))
        pssm = ctx.enter_context(tc.tile_pool(name="pssm", bufs=2, space="PSUM"))

        # ---- prefix loads, AHEAD of the x stream on the sync queue: the
        # tiny tensors first, then the two weight matrices the logits
        # stationary depends on.  (One chunk-major DMA per weight.) ----
        ctx_sb = singles.tile([P, CCH], BF16, tag="ctx", name="ctx")
        nc.sync.dma_start(ctx_sb[:], ctxd[:])
        bqk_sb = singles.tile([P, CCH], F32, tag="bqk", name="bqk")
        nc.sync.dma_start(bqk_sb[:], bqkd[:])
        wqk_sb = singles.tile([P, CCH * C], BF16, tag="wqk", name="wqk")
        nc.sync.dma_start(wqk_sb[:], wqkd[:])

        ones128 = singles.tile([P, P], BF16, tag="ones128")
        nc.vector.memset(ones128[:], 1.0)

        qks_sb = singles.tile([P, CCH], F32, tag="qks")
        qksb = [singles.tile([P, P], BF16, tag=f"qksb{cc}", name=f"qksb{cc}") for cc in range(CCH)]
        pooled_sb = singles.tile([P, CCH], BF16, tag="pooled")
        pooled_f32 = singles.tile([P, CCH], F32, tag="pooledf")
        gate_sb = singles.tile([P, CCH], F32, tag="gate")
        zcols = singles.tile([P, G * NH], F32, tag="zcols")
        pcols = [singles.tile([P, G], F32, tag=f"pcols{cc}", name=f"pcols{cc}") for cc in range(CCH)]
        z_sb = singles.tile([P, 1], F32, tag="z")
        rz_sb = singles.tile([P, 1], F32, tag="rz")

        # ---- qks = ctx @ Wqk + bqk (pre-scaled); qksB replicated 128x ----
        for cc in range(CCH):
            pqk = pssm.tile([P, 1], F32, tag="pssm", name="pssm_t")
            for dc in range(CCH):
                nc.tensor.matmul(
                    pqk[:], wqk_sb[:, dc * C + cc * P:dc * C + (cc + 1) * P],
                    ctx_sb[:, dc:dc + 1],
                    start=(dc == 0), stop=(dc == CCH - 1),
                )
            nc.vector.tensor_add(qks_sb[:, cc:cc + 1], pqk[:], bqk_sb[:, cc:cc + 1])
            nc.vector.tensor_scalar_mul(qksb[cc][:], ones128[:], qks_sb[:, cc:cc + 1])

        # ---- fused pass A+B: logits (all-partition rows) -> exp -> pooled ----
        # DVE: 3 fused STTs + 1 TT-mult per group; ACT: 2 exps + 1 reduce
        # (lagged one group so exps never queue behind it).
        x_tiles = {}
        pr_tiles = {}

        def _act_reduce(g):
            for cc in (2, 3):
                sa = scra.tile([P, GW], BF16, tag="scra", name="scra_t")
                nc.scalar.activation(
                    sa[:], pr_tiles[(cc, g)][:], AF.Copy,
                    accum_out=pcols[cc][:, g:g + 1],
                )

        for g in range(G):
            for cc in range(CCH):
                t = xt.tile([P, GW], BF16, tag="x", name="x_t")
                nc.sync.dma_start(t[:], xd[cc * P:(cc + 1) * P, g * GW:(g + 1) * GW])
                x_tiles[(cc, g)] = t
            p_t = psb.tile([P, GW], BF16, tag="p", name="p_t")
            for h in range(NH):
                gh = g * NH + h
                plog = pslog.tile([P, PGW], F32, tag="plog", name="plog_t")
                for s in range(PGW // 512):
                    for cc in range(CCH):
                        nc.tensor.matmul(
                            plog[:, s * 512:(s + 1) * 512],
                            qksb[cc][:],
                            x_tiles[(cc, g)][:, h * PGW + s * 512:h * PGW + (s + 1) * 512],
                            start=(cc == 0), stop=(cc == CCH - 1),
                        )
                nc.scalar.activation(
                    p_t[:, h * PGW:(h + 1) * PGW], plog[:], AF.Exp,
                    accum_out=zcols[:, gh:gh + 1],
                )
            if g > 0:
                _act_reduce(g - 1)
            for cc in (0, 1):
                sc = scr.tile([P, GW], BF16, tag="scrv", name="scr_t")
                nc.vector.scalar_tensor_tensor(
                    sc[:], x_tiles[(cc, g)][:], 1.0, p_t[:],
                    op0=OP.mult, op1=OP.mult,
                    accum_out=pcols[cc][:, g:g + 1],
                )
            for cc in (2, 3):
                pr = prods.tile([P, GW], BF16, tag=f"pr{cc}", name=f"pr{cc}_t")
                nc.vector.tensor_mul(pr[:], x_tiles[(cc, g)][:], p_t[:])
                pr_tiles[(cc, g)] = pr
        _act_reduce(G - 1)

        # ---- late loads: Wv.T / bv, queued behind the x stream ----
        wvt_sb = singles.tile([P, CCH * C], BF16, tag="wvt", name="wvt")
        nc.scalar.dma_start(wvt_sb[:], wvtd[:])
        bv_sb = singles.tile([P, CCH], F32, tag="bv", name="bv")
        nc.scalar.dma_start(bv_sb[:], bvd[:])

        # ---- finalize: Z, pooled, gate = (Wv @ pooled)/Z + bv ----
        nc.vector.reduce_sum(z_sb[:], zcols[:], axis=mybir.AxisListType.X)
        nc.vector.reciprocal(rz_sb[:], z_sb[:])
        for cc in range(CCH):
            nc.vector.reduce_sum(
                pooled_f32[:, cc:cc + 1], pcols[cc][:], axis=mybir.AxisListType.X
            )
        nc.vector.tensor_copy(pooled_sb[:], pooled_f32[:])
        for oc in range(CCH):
            pg = pssm.tile([P, 1], F32, tag="pssm", name="pssm_t")
            for cc in range(CCH):
                nc.tensor.matmul(
                    pg[:], wvt_sb[:, cc * C + oc * P:cc * C + (oc + 1) * P],
                    pooled_sb[:, cc:cc + 1],
                    start=(cc == 0), stop=(cc == CCH - 1),
                )
            nc.vector.scalar_tensor_tensor(
                gate_sb[:, oc:oc + 1], pg[:], rz_sb[:], bv_sb[:, oc:oc + 1],
                op0=OP.mult, op1=OP.add,
            )

        # ---- pass C: out = x * gate (all of x is still resident in SBUF) ----
        for idx in range(G * CCH):
            g, cc = divmod(idx, CCH)
            o = outp.tile([P, GW], BF16, tag="o", name="o_t")
            nc.vector.tensor_scalar_mul(o[:], x_tiles[(cc, g)][:], gate_sb[:, cc:cc + 1])
            eng = nc.sync if idx % 2 == 0 else nc.scalar
            eng.dma_start(outd[cc * P:(cc + 1) * P, g * GW:(g + 1) * GW], o[:])

    nc.compile()
    return nc


_NC = None


def _get_nc():
    global _NC
    if _NC is None:
        _NC = _build_kernel()
    return _NC


def _chunk_major(w):
    # w_c[p, j*C + k] = w[j*128 + p, k]
    w = np.asarray(w, dtype=np.float32).reshape(CCH, P, C)
    return np.ascontiguousarray(w.transpose(1, 0, 2).reshape(P, CCH * C))


def _make_in_maps(x, context, Wq, bq, Wk, bk, Wv, bv):
    bf = ml_dtypes.bfloat16
    x = np.asarray(x, dtype=np.float32).reshape(B, C, HW).astype(bf)
    Wq = np.asarray(Wq, dtype=np.float32)
    Wk = np.asarray(Wk, dtype=np.float32)
    wqk = _chunk_major(Wq.T @ Wk * SCALE).astype(bf)
    bqk = (np.asarray(bq, dtype=np.float32) @ Wk) * SCALE
    wvt = _chunk_major(np.asarray(Wv, dtype=np.float32).T).astype(bf)
    bqkc = np.ascontiguousarray(bqk.reshape(CCH, P).T)
    bvc = np.ascontiguousarray(np.asarray(bv, dtype=np.float32).reshape(CCH, P).T)
    context = np.asarray(context, dtype=np.float32)
    in_maps = []
    for b in range(NCORES):
        ctxc = np.ascontiguousarray(context[b].reshape(CCH, P).T).astype(bf)
        in_maps.append({
            "xb": x[b],
            "ctxc": ctxc,
            "wqk": wqk,
            "wvt": wvt,
            "bqkc": bqkc,
            "bvc": bvc,
        })
    return in_maps


def run_spmd(x, context, Wq, bq, Wk, bk, Wv, bv, **spmd_kwargs):
    """Run on 8 NeuronCores; returns (output (B,C,H,W) f32, BassKernelResults)."""
    nc = _get_nc()
    in_maps = _make_in_maps(x, context, Wq, bq, Wk, bk, Wv, bv)
    res = run_bass_kernel_spmd(nc, in_maps, list(range(NCORES)), **spmd_kwargs)
    out = np.stack([
        np.asarray(res.results[b]["out"]).astype(np.float32).reshape(C, H, W)
        for b in range(NCORES)
    ])
    return out, res


def kernel(x, context, Wq, bq, Wk, bk, Wv, bv):
    out, _ = run_spmd(x, context, Wq, bq, Wk, bk, Wv, bv)
    return out
